# revision 15
# baseline (speedup 1.0000x reference)
"""Trainium2 Bass kernel for the CompressiveMemory module (8-core SPMD).

Contract: kernel(**inputs) takes the FULL unsharded inputs
(queries/keys/values (2,4096,1024), memories (4,1024,1024),
memory_norms (4,1024), all fp32) and returns the full outputs
(out (2,4096,1024), new_memory (1024,1024), new_memory_norm (1024,)).

Strategy: data-parallel split of the 8192 (batch*seq) rows across the 8
NeuronCores (1024 rows each), memory state replicated. Per core:
  sigma = elu(x)+1 = exp(min(x,0)) + max(x,0)
  retrieve: a_m = sigma_q @ mem_m (f32r matmuls), qnorm = sigma_q @ norms^T,
            out = sum_m softmax_m(rel)[m]/qnorm[:,m] * a_m
  update:   ru = sigma_k @ mem0, dv = v - ru/knorm,
            mu_part = sigma_k^T @ dv, colsum_part = 1^T sigma_k
Cross-core reductions:
  - a tiny AllReduce (relevance partials + sigma_k column sums, 4KB) right
    after phase A unblocks the softmax weights and new_memory_norm early;
  - the delta-rule update partial (4MB) uses ReduceScatter — each core
    receives only its 128-row shard of the summed update, finalizes
    new_memory for those rows, and the host concatenates the shards.
    (An AllReduce here ran the mesh algorithm = 7x the wire traffic and
    starved phase C's DMA for ~70us.)
Phase C streams memory matrices m=1..3 from HBM; m=1 is prefetched into
SBUF before the ReduceScatter is triggered so the PE bridges the RS window
with the m=0 (cached) and m=1 (prefetched) matmul groups.
"""

import os
import sys
from contextlib import ExitStack

for _p in ("/opt/trn_rl_repo", "/root/.axon_site/_ro/trn_rl_repo"):
    if os.path.isdir(_p) and _p not in sys.path:
        sys.path.insert(0, _p)

import numpy as np

import concourse.bass as bass
import concourse.tile as tile
import concourse.mybir as mybir
from concourse import bacc, bass_utils
from concourse.bass_interp import get_hw_module
from concourse.masks import make_identity

F32 = mybir.dt.float32
BF16 = mybir.dt.bfloat16
F32R = mybir.dt.float32r
AF = mybir.ActivationFunctionType
ALU = mybir.AluOpType

NC = 8          # cores
P = 128         # partitions
D = 1024        # feature dim
M = 4           # memory slots
B, S = 2, 4096
R = (B * S) // NC      # rows per core
ST = R // P            # 8 s-tiles per core
KD = D // P            # 8 contraction-dim chunks
H = 512                # matmul free dim / half width
EB = D // H            # 2 e-blocks
INV_BS = 1.0 / (B * S)
INV_S = 1.0 / S
INV_B = 1.0 / B


def build_program():
    nc = bacc.Bacc("TRN2", target_bir_lowering=False, debug=False, num_devices=NC)

    aps = {}
    aps["q"] = nc.dram_tensor("q", [R, D], F32, kind="ExternalInput").ap()
    aps["k"] = nc.dram_tensor("k", [R, D], F32, kind="ExternalInput").ap()
    aps["v"] = nc.dram_tensor("v", [R, D], F32, kind="ExternalInput").ap()
    aps["mems"] = nc.dram_tensor("mems", [M, D, D], F32, kind="ExternalInput").ap()
    aps["norms"] = nc.dram_tensor("norms", [M, D], F32, kind="ExternalInput").ap()
    aps["bmask"] = nc.dram_tensor("bmask", [1, 2], F32, kind="ExternalInput").ap()
    aps["m0row"] = nc.dram_tensor("m0row", [P, D], F32, kind="ExternalInput").ap()

    aps["out_rows"] = nc.dram_tensor("out_rows", [R, D], F32, kind="ExternalOutput").ap()
    aps["nmem_shard"] = nc.dram_tensor("nmem_shard", [P, D], F32, kind="ExternalOutput").ap()
    aps["new_norm"] = nc.dram_tensor("new_norm", [1, D], F32, kind="ExternalOutput").ap()

    with tile.TileContext(nc) as tc:
        _build(nc, tc, aps)
    nc.compile()
    nc.m = get_hw_module(nc.m)
    return nc


def _build(nc, tc, aps):
    q_ap, k_ap, v_ap, mems_ap, norms_ap, bmask_ap, m0row_ap = (
        aps["q"], aps["k"], aps["v"], aps["mems"], aps["norms"], aps["bmask"],
        aps["m0row"])
    out_ap, nmem_ap, nnorm_ap = aps["out_rows"], aps["nmem_shard"], aps["new_norm"]
    rg = [list(range(NC))]
    with ExitStack() as stack:
        singles = stack.enter_context(tc.tile_pool(name="singles", bufs=1))
        persist = stack.enter_context(tc.tile_pool(name="persist", bufs=1))
        dram = stack.enter_context(tc.tile_pool(name="dram", bufs=1, space="DRAM"))

        # ---------- constants ----------
        ident_f = singles.tile([P, P], F32, tag="ident_f", name="ident_f")
        make_identity(nc, ident_f)
        ident_r = singles.tile([P, P], F32R, tag="ident_r", name="ident_r")
        nc.vector.tensor_copy(ident_r, ident_f)

        ones_f = singles.tile([P, 1], F32, tag="ones_f", name="ones_f")
        nc.vector.memset(ones_f, 1.0)
        ones_r = singles.tile([P, 1], F32R, tag="ones_r", name="ones_r")
        nc.vector.tensor_copy(ones_r, ones_f)

        bmask_sb = singles.tile([P, 2], F32, tag="bmask_sb", name="bmask_sb")
        rel8 = singles.tile([1, M, 2], F32, tag="rel8", name="rel8")
        cs_sbs = [singles.tile([1, H], F32, tag=f"cs_sb{h}", name=f"cs_sb{h}")
                  for h in range(2)]
        # norms^T tiles [d-part, m]; loaded inside phase A (slow strided DMA
        # must not sit in front of the input-tile queue)
        normsT_f = [singles.tile([P, M], F32, tag=f"nTf{kd}", name=f"nTf{kd}")
                    for kd in range(KD)]
        normsT = [singles.tile([P, M], F32R, tag=f"nTr{kd}", name=f"nTr{kd}")
                  for kd in range(KD)]

        # persistent across A..C
        mem0_r = [persist.tile([P, D], F32R, tag=f"mem0_r{kd}", name=f"mem0_r{kd}")
                  for kd in range(KD)]
        sqT = [persist.tile([P, D], F32R, tag=f"sqT{st}", name=f"sqT{st}")
               for st in range(ST)]
        qnorm_r = [persist.tile([P, M], F32R, tag=f"qnorm{st}", name=f"qnorm{st}")
                   for st in range(ST)]
        c_all = [persist.tile([P, M], F32, tag=f"c_all{st}", name=f"c_all{st}")
                 for st in range(ST)]
        # m=1 eb0 weight prefetch (filled after phase A, used to bridge the RS)
        w1c = [persist.tile([P, H], F32R, tag=f"w1c{kd}", name=f"w1c{kd}")
               for kd in range(KD)]

        # collective bounce buffers
        ar1_in = dram.tile([1, 8 + D], F32, tag="ar1_in", name="ar1_in")
        ar1_out = dram.tile([1, 8 + D], F32, tag="ar1_out", name="ar1_out", addr_space="Shared")
        rs_in = dram.tile([D, D], F32, tag="rs_in", name="rs_in")
        rs_out = dram.tile([P, D], F32, tag="rs_out", name="rs_out")

        with ExitStack() as ab_stack:
            # tensors that live through phases A+B
            p_ab = ab_stack.enter_context(tc.tile_pool(name="p_ab", bufs=1))
            sk_nat = [p_ab.tile([P, D], F32R, tag=f"sk_nat{st}", name=f"sk_nat{st}")
                      for st in range(ST)]
            dv = [p_ab.tile([P, D], F32R, tag=f"dv{st}", name=f"dv{st}")
                  for st in range(ST)]

            # ========== phase A: sigma, transposes, qnorm/knorm, ru, dv ==========
            with tc.tile_pool(name="a_in", bufs=3) as a_in, \
                 tc.tile_pool(name="a_tmp", bufs=7) as a_tmp, \
                 tc.tile_pool(name="a_skt", bufs=2) as a_skt, \
                 tc.tile_pool(name="a_sm", bufs=4) as a_sm, \
                 tc.tile_pool(name="ps_tp", bufs=2, space="PSUM") as ps_tp, \
                 tc.tile_pool(name="ps_ru", bufs=2, space="PSUM") as ps_ru, \
                 tc.tile_pool(name="ps_qn", bufs=1, space="PSUM") as ps_qn:

                for st in range(ST):
                    rs = slice(st * P, (st + 1) * P)
                    skT = a_skt.tile([P, D], F32R, tag="skT", name="skT")
                    # sigma(x) = exp(-relu(-x)) + relu(x); relu on ACT (scale=+-1),
                    # exp on ACT, adds on DVE. Function-batched to limit ACT
                    # table swaps: all Relus, then all Exps per s-tile.
                    k_ts, q_ts, tpqs = [], [], []
                    kneg, kpos, qneg, qpos = [], [], [], []
                    for h in range(2):
                        hs = slice(h * H, (h + 1) * H)
                        k_t = a_in.tile([P, H], F32, tag="k_t", name="k_t")
                        nc.sync.dma_start(k_t, k_ap[rs, hs])
                        k_ts.append(k_t)
                        q_t = a_in.tile([P, H], F32, tag="q_t", name="q_t")
                        nc.sync.dma_start(q_t, q_ap[rs, hs])
                        q_ts.append(q_t)
                    # transposes of raw q (PE) into psum
                    for h in range(2):
                        tpq = ps_tp.tile([P, H], F32, tag="tpq", name="tpq")
                        for j in range(4):
                            nc.tensor.transpose(
                                tpq[:, j * P:(j + 1) * P],
                                q_ts[h][:, j * P:(j + 1) * P], ident_f)
                        tpqs.append(tpq)
                    # k group on ACT: relu(-x), relu(x), exp(-r)
                    for h in range(2):
                        kn = a_tmp.tile([P, H], F32, tag="tmp", name="kn")
                        nc.scalar.activation(kn, k_ts[h], AF.Relu, scale=-1.0)
                        kneg.append(kn)
                        kp = a_tmp.tile([P, H], F32, tag="tmp", name="kp")
                        nc.scalar.activation(kp, k_ts[h], AF.Relu)
                        kpos.append(kp)
                    for h in range(2):
                        hs = slice(h * H, (h + 1) * H)
                        ke = a_tmp.tile([P, H], F32, tag="tmp", name="ke")
                        nc.scalar.activation(ke, kneg[h], AF.Exp, scale=-1.0)
                        nc.vector.tensor_tensor(sk_nat[st][:, hs], ke, kpos[h],
                                                ALU.add)
                    # q group: min/max on DVE (psum reads), exp on ACT
                    for h in range(2):
                        hs = slice(h * H, (h + 1) * H)
                        qmin = a_tmp.tile([P, H], F32, tag="tmp", name="qmin")
                        nc.vector.tensor_scalar_min(qmin, tpqs[h], 0.0)
                        qmax = a_tmp.tile([P, H], F32, tag="tmp", name="qmax")
                        nc.vector.tensor_scalar_max(qmax, tpqs[h], 0.0)
                        qe = a_tmp.tile([P, H], F32, tag="tmp", name="qe")
                        nc.scalar.activation(qe, qmin, AF.Exp)
                        nc.vector.tensor_tensor(sqT[st][:, hs], qe, qmax,
                                                ALU.add)
                    # --- sigma_k transposed (PE transpose f32r, DVE copyback) ---
                    for h in range(2):
                        tps = ps_tp.tile([P, H], F32R, tag="tps", name="tps")
                        for j in range(4):
                            kd = h * 4 + j
                            nc.tensor.transpose(
                                tps[:, j * P:(j + 1) * P],
                                sk_nat[st][:, kd * P:(kd + 1) * P], ident_r)
                        nc.vector.tensor_copy(skT[:, h * H:(h + 1) * H], tps)

                    if st == 0:
                        nc.sync.dma_start(out=bmask_sb,
                                          in_=bmask_ap.to_broadcast((P, 2)))
                        for kd in range(KD):
                            with nc.allow_non_contiguous_dma(reason="norms^T"):
                                nc.sync.dma_start(
                                    normsT_f[kd],
                                    norms_ap[:, kd * P:(kd + 1) * P]
                                    .rearrange("m d -> d m"))
                            nc.vector.tensor_copy(normsT[kd], normsT_f[kd])
                        # mem0 load + round, after st0's inputs so PE starts early
                        for kd in range(KD):
                            for h in range(2):
                                hs = slice(h * H, (h + 1) * H)
                                m0f = a_tmp.tile([P, H], F32, tag="tmp", name="m0f")
                                nc.sync.dma_start(
                                    m0f, mems_ap[0, kd * P:(kd + 1) * P, hs])
                                nc.vector.tensor_copy(mem0_r[kd][:, hs], m0f)
                    if st == 1:
                        # m=1 eb0 weight prefetch (bridges the RS window in C)
                        for kd in range(KD):
                            wf = a_tmp.tile([P, H], F32, tag="tmp", name="w1pf")
                            nc.sync.dma_start(
                                wf, mems_ap[1, kd * P:(kd + 1) * P, 0:H])
                            nc.scalar.activation(w1c[kd], wf, AF.Copy)

                    # --- qnorm / knorm ---
                    qn_ps = ps_qn.tile([P, M], F32, tag="qn", name="qn_ps")
                    for kd in range(KD):
                        cs = slice(kd * P, (kd + 1) * P)
                        nc.tensor.matmul(qn_ps, sqT[st][:, cs], normsT[kd],
                                         start=(kd == 0), stop=(kd == KD - 1))
                    kn_ps = ps_qn.tile([P, M], F32, tag="kn", name="kn_ps")
                    for kd in range(KD):
                        cs = slice(kd * P, (kd + 1) * P)
                        nc.tensor.matmul(kn_ps, skT[:, cs], normsT[kd],
                                         start=(kd == 0), stop=(kd == KD - 1))
                    nc.vector.tensor_copy(qnorm_r[st], qn_ps)
                    knr = a_sm.tile([P, 1], F32, tag="knr", name="knr")
                    nc.vector.reciprocal(knr, kn_ps[:, 0:1])

                    # --- retrieved_unnorm + delta_v ---
                    for eb in range(EB):
                        es = slice(eb * H, (eb + 1) * H)
                        v_t = a_in.tile([P, H], F32, tag="v_t", name="v_t")
                        nc.sync.dma_start(v_t, v_ap[rs, es])
                        ru_ps = ps_ru.tile([P, H], F32, tag="ru", name="ru_ps")
                        for kd in range(KD):
                            cs = slice(kd * P, (kd + 1) * P)
                            nc.tensor.matmul(ru_ps, skT[:, cs], mem0_r[kd][:, es],
                                             start=(kd == 0), stop=(kd == KD - 1))
                        dvt = a_tmp.tile([P, H], F32, tag="tmp", name="dvt")
                        nc.scalar.activation(dvt, ru_ps, AF.Copy, scale=knr)
                        nc.vector.tensor_tensor(dv[st][:, es], v_t, dvt, ALU.subtract)


            # ===== rel partials + colsum + small AllReduce (early) =====
            with tc.tile_pool(name="ps_rel", bufs=1, space="PSUM") as ps_rel, \
                 tc.tile_pool(name="ps_cs", bufs=1, space="PSUM") as ps_cs:
                rel_ps = ps_rel.tile([1, M], F32, tag="rel", name="rel_ps")
                for st in range(ST):
                    nc.tensor.matmul(rel_ps, ones_r, qnorm_r[st],
                                     start=(st == 0), stop=(st == ST - 1))
                nc.vector.tensor_tensor(
                    rel8,
                    rel_ps[0:1, :, None].to_broadcast((1, M, 2)),
                    bmask_sb[0:1, None, :].to_broadcast((1, M, 2)),
                    ALU.mult)
                nc.sync.dma_start(
                    ar1_in[0:1, 0:8].rearrange("o (f t) -> o f t", t=2), rel8)
                for h in range(2):
                    es = slice(h * H, (h + 1) * H)
                    cs_ps = ps_cs.tile([1, H], F32, tag="cs", name="cs_ps")
                    for st in range(ST):
                        nc.tensor.matmul(cs_ps, ones_r, sk_nat[st][:, es],
                                         start=(st == 0), stop=(st == ST - 1))
                    nc.vector.tensor_copy(cs_sbs[h], cs_ps)
                    nc.sync.dma_start(ar1_in[0:1, 8 + h * H:8 + (h + 1) * H],
                                      cs_sbs[h])
                nc.gpsimd.collective_compute(
                    "AllReduce", ALU.add, replica_groups=rg,
                    ins=[ar1_in.opt()], outs=[ar1_out.opt()])

            # ========== phase B: mu partials + ReduceScatter ==========
            with tc.tile_pool(name="b_stage", bufs=3) as b_stage, \
                 tc.tile_pool(name="ps_mu", bufs=2, space="PSUM") as ps_mu:
                for kd in range(KD):
                    cs_k = slice(kd * P, (kd + 1) * P)
                    for eb in range(EB):
                        es = slice(eb * H, (eb + 1) * H)
                        mu_ps = ps_mu.tile([P, H], F32, tag="mu", name="mu_ps")
                        for st in range(ST):
                            nc.tensor.matmul(mu_ps, sk_nat[st][:, cs_k],
                                             dv[st][:, es],
                                             start=(st == 0), stop=(st == ST - 1))
                        stg = b_stage.tile([P, H], F32, tag="stg", name="stg")
                        nc.vector.tensor_copy(stg, mu_ps)
                        nc.sync.dma_start(rs_in[kd * P:(kd + 1) * P, es], stg)
                nc.gpsimd.collective_compute(
                    "ReduceScatter", ALU.add, replica_groups=rg,
                    ins=[rs_in.opt()], outs=[rs_out.opt()])

        # ========== weights from AR1 (redundant on all partitions), new_norm ====
        with tc.tile_pool(name="w_sb", bufs=1) as wp:
            g8 = wp.tile([P, M, 2], F32, tag="g8", name="g8")
            nc.sync.dma_start(
                g8, ar1_out[0:1, 0:8].rearrange("o (f t) -> o f t", t=2)
                .to_broadcast((P, M, 2)))
            t0 = wp.tile([P, M], F32, tag="t0", name="t0")
            nc.vector.tensor_scalar_mul(t0, g8[:, :, 0], bmask_sb[:, 0:1])
            t1 = wp.tile([P, M], F32, tag="t1", name="t1")
            nc.vector.tensor_scalar_mul(t1, g8[:, :, 1], bmask_sb[:, 1:2])
            rsum = wp.tile([P, M], F32, tag="rsum", name="rsum")
            nc.vector.tensor_tensor(rsum, t0, t1, ALU.add)
            mx = wp.tile([P, 1], F32, tag="mx", name="mx")
            nc.vector.tensor_reduce(mx, rsum, axis=mybir.AxisListType.X, op=ALU.max)
            negmx = wp.tile([P, 1], F32, tag="negmx", name="negmx")
            nc.vector.tensor_scalar_mul(negmx, mx, -INV_S)
            ex = wp.tile([P, M], F32, tag="ex", name="ex")
            nc.scalar.activation(ex, rsum, AF.Exp, bias=negmx, scale=INV_S)
            sm = wp.tile([P, 1], F32, tag="sm", name="sm")
            nc.vector.tensor_reduce(sm, ex, axis=mybir.AxisListType.X, op=ALU.add)
            rs_ = wp.tile([P, 1], F32, tag="rs_", name="rs_")
            nc.vector.reciprocal(rs_, sm)
            w128 = wp.tile([P, M], F32, tag="w128", name="w128")
            nc.vector.tensor_scalar_mul(w128, ex, rs_)
            for st in range(ST):
                rq = wp.tile([P, M], F32, tag=f"rq{st}", name=f"rq{st}")
                nc.vector.reciprocal(rq, qnorm_r[st].bitcast(F32))
                nc.vector.tensor_tensor(c_all[st], rq, w128, ALU.mult)
            # new_norm = norms[0] + colsum/B  (from AR1)
            csg = wp.tile([1, D], F32, tag="csg", name="csg")
            nc.sync.dma_start(csg, ar1_out[0:1, 8:8 + D])
            n0 = wp.tile([1, D], F32, tag="n0", name="n0")
            nc.sync.dma_start(n0, norms_ap[0:1, :])
            csh = wp.tile([1, D], F32, tag="csh", name="csh")
            nc.vector.tensor_scalar_mul(csh, csg, INV_B)
            nn = wp.tile([1, D], F32, tag="nn", name="nn")
            nc.vector.tensor_tensor(nn, csh, n0, ALU.add)
            nc.sync.dma_start(nnorm_ap, nn)

        # ========== phase C: retrieve matmuls + combine (m outer) ==========
        with tc.tile_pool(name="c_out", bufs=1) as c_out, \
             tc.tile_pool(name="c_wf", bufs=3) as c_wf, \
             tc.tile_pool(name="c_wr", bufs=10) as c_wr, \
             tc.tile_pool(name="c_tmp", bufs=3) as c_tmp, \
             tc.tile_pool(name="c_d", bufs=1) as c_d, \
             tc.tile_pool(name="ps_am", bufs=4, space="PSUM") as ps_am:
            out_sb = [c_out.tile([P, D], F32, tag=f"out_sb{st}", name=f"out_sb{st}")
                      for st in range(ST)]

            def mm_group_eb(m, eb, rhs_kd_tiles):
                es = slice(eb * H, (eb + 1) * H)
                for st in range(ST):
                    am_ps = ps_am.tile([P, H], F32, tag="am", name="am_ps")
                    for kd in range(KD):
                        cs = slice(kd * P, (kd + 1) * P)
                        nc.tensor.matmul(am_ps, sqT[st][:, cs], rhs_kd_tiles[kd],
                                         start=(kd == 0), stop=(kd == KD - 1))
                    if m == 0:
                        nc.vector.tensor_scalar_mul(
                            out_sb[st][:, es], am_ps, c_all[st][:, 0:1])
                    else:
                        tmp = c_tmp.tile([P, H], F32, tag="ctmp", name="ctmp")
                        nc.vector.tensor_scalar_mul(
                            tmp, am_ps, c_all[st][:, m:m + 1])
                        nc.vector.tensor_tensor(
                            out_sb[st][:, es], out_sb[st][:, es], tmp, ALU.add)
                    if m == M - 1 and eb == EB - 1:
                        nc.sync.dma_start(out_ap[st * P:(st + 1) * P, :],
                                          out_sb[st])

            def mm_group(m, rhs_tiles):
                for eb in range(EB):
                    mm_group_eb(m, eb, [rhs_tiles[kd * EB + eb] for kd in range(KD)])

            # m=0 from cached mem0_r
            mm_group(0, [mem0_r[kd][:, slice(eb * H, (eb + 1) * H)]
                         for kd in range(KD) for eb in range(EB)])
            # m=1 eb0 from prefetched cache
            mm_group_eb(1, 0, w1c)

            def stream_group(m, eb):
                es = slice(eb * H, (eb + 1) * H)
                rhs_tiles = []
                for kd in range(KD):
                    wf = c_wf.tile([P, H], F32, tag="wf", name="wf")
                    nc.sync.dma_start(wf, mems_ap[m, kd * P:(kd + 1) * P, es])
                    wr = c_wr.tile([P, H], F32R, tag="wr", name="wr")
                    nc.scalar.activation(wr, wf, AF.Copy)
                    rhs_tiles.append(wr)
                mm_group_eb(m, eb, rhs_tiles)

            stream_group(1, 1)

            # m=2,3 streamed per e-block (post-RS DMA bandwidth)
            for m in (2, 3):
                for eb in range(EB):
                    stream_group(m, eb)

            # finalize new_memory shard (needs RS result; tiny)
            g_t = c_d.tile([P, D], F32, tag="g_t", name="g_t")
            nc.sync.dma_start(g_t, rs_out)
            m0r_t = c_d.tile([P, D], F32, tag="m0r_t", name="m0r_t")
            nc.sync.dma_start(m0r_t, m0row_ap)
            nm = c_d.tile([P, D], F32, tag="nm", name="nm")
            nc.vector.tensor_scalar_mul(nm, g_t, INV_BS)
            nm2 = c_d.tile([P, D], F32, tag="nm2", name="nm2")
            nc.vector.tensor_tensor(nm2, nm, m0r_t, ALU.add)
            nc.sync.dma_start(nmem_ap, nm2)


_CACHED = None


def _get_program():
    global _CACHED
    if _CACHED is None:
        _CACHED = build_program()
    return _CACHED


def _make_in_maps(queries, keys, values, memories, memory_norms):
    Q = np.ascontiguousarray(queries.reshape(B * S, D), dtype=np.float32)
    K = np.ascontiguousarray(keys.reshape(B * S, D), dtype=np.float32)
    V = np.ascontiguousarray(values.reshape(B * S, D), dtype=np.float32)
    mems = np.ascontiguousarray(memories, dtype=np.float32)
    norms = np.ascontiguousarray(memory_norms, dtype=np.float32)
    in_maps = []
    for c in range(NC):
        rows = slice(c * R, (c + 1) * R)
        bm = np.zeros((1, 2), np.float32)
        bm[0, c // (NC // B)] = 1.0
        in_maps.append({
            "q": Q[rows], "k": K[rows], "v": V[rows],
            "mems": mems, "norms": norms, "bmask": bm,
            "m0row": mems[0, c * P:(c + 1) * P, :],
        })
    return in_maps


def run(queries, keys, values, memories, memory_norms, trace=False, **trace_kwargs):
    nc = _get_program()
    in_maps = _make_in_maps(queries, keys, values, memories, memory_norms)
    res = bass_utils.run_bass_kernel_spmd(
        nc, in_maps, core_ids=list(range(NC)), trace=trace, **trace_kwargs)
    out = np.concatenate([res.results[c]["out_rows"] for c in range(NC)],
                         axis=0).reshape(B, S, D)
    new_memory = np.concatenate([res.results[c]["nmem_shard"] for c in range(NC)],
                                axis=0)
    new_norm = res.results[0]["new_norm"][0]
    return (out, new_memory, new_norm), res


def kernel(queries, keys, values, memories, memory_norms):
    (out, new_memory, new_norm), _ = run(
        queries, keys, values, memories, memory_norms, trace=False)
    return out, new_memory, new_norm


# revision 16
# speedup vs baseline: 1.2337x; 1.2337x over previous
"""Trainium2 Bass kernel for the CompressiveMemory module (8-core SPMD).

Contract: kernel(**inputs) takes the FULL unsharded inputs
(queries/keys/values (2,4096,1024), memories (4,1024,1024),
memory_norms (4,1024), all fp32) and returns the full outputs
(out (2,4096,1024), new_memory (1024,1024), new_memory_norm (1024,)).

Strategy: data-parallel split of the 8192 (batch*seq) rows across the 8
NeuronCores (1024 rows each), memory state replicated. Per core:
  sigma = elu(x)+1 = exp(min(x,0)) + max(x,0)
  retrieve: a_m = sigma_q @ mem_m (f32r matmuls), qnorm = sigma_q @ norms^T,
            out = sum_m softmax_m(rel)[m]/qnorm[:,m] * a_m
  update:   ru = sigma_k @ mem0, dv = v - ru/knorm,
            mu_part = sigma_k^T @ dv, colsum_part = 1^T sigma_k
Cross-core reductions:
  - a tiny AllReduce (relevance partials + sigma_k column sums, 4KB) right
    after phase A unblocks the softmax weights and new_memory_norm early;
  - the delta-rule update partial (4MB) uses ReduceScatter — each core
    receives only its 128-row shard of the summed update, finalizes
    new_memory for those rows, and the host concatenates the shards.
    (An AllReduce here ran the mesh algorithm = 7x the wire traffic and
    starved phase C's DMA for ~70us.)
Phase C streams memory matrices m=1..3 from HBM; m=1 is prefetched into
SBUF before the ReduceScatter is triggered so the PE bridges the RS window
with the m=0 (cached) and m=1 (prefetched) matmul groups.
"""

import os
import sys
from contextlib import ExitStack

for _p in ("/opt/trn_rl_repo", "/root/.axon_site/_ro/trn_rl_repo"):
    if os.path.isdir(_p) and _p not in sys.path:
        sys.path.insert(0, _p)

import numpy as np

import concourse.bass as bass
import concourse.tile as tile
import concourse.mybir as mybir
from concourse import bacc, bass_utils
from concourse.bass_interp import get_hw_module
from concourse.masks import make_identity

F32 = mybir.dt.float32
BF16 = mybir.dt.bfloat16
F32R = mybir.dt.float32r
AF = mybir.ActivationFunctionType
ALU = mybir.AluOpType

NC = 8          # cores
P = 128         # partitions
D = 1024        # feature dim
M = 4           # memory slots
B, S = 2, 4096
R = (B * S) // NC      # rows per core
ST = R // P            # 8 s-tiles per core
KD = D // P            # 8 contraction-dim chunks
H = 512                # matmul free dim / half width
EB = D // H            # 2 e-blocks
INV_BS = 1.0 / (B * S)
INV_S = 1.0 / S
INV_B = 1.0 / B


def build_program():
    nc = bacc.Bacc("TRN2", target_bir_lowering=False, debug=False, num_devices=NC)

    aps = {}
    aps["q"] = nc.dram_tensor("q", [R, D], F32, kind="ExternalInput").ap()
    aps["k"] = nc.dram_tensor("k", [R, D], F32, kind="ExternalInput").ap()
    aps["v"] = nc.dram_tensor("v", [R, D], F32, kind="ExternalInput").ap()
    aps["mems"] = nc.dram_tensor("mems", [M, D, D], F32, kind="ExternalInput").ap()
    aps["norms"] = nc.dram_tensor("norms", [M, D], F32, kind="ExternalInput").ap()
    aps["bmask"] = nc.dram_tensor("bmask", [1, 2], F32, kind="ExternalInput").ap()
    aps["m0row"] = nc.dram_tensor("m0row", [P, D], F32, kind="ExternalInput").ap()

    aps["out_rows"] = nc.dram_tensor("out_rows", [R, D], F32, kind="ExternalOutput").ap()
    aps["nmem_shard"] = nc.dram_tensor("nmem_shard", [P, D], F32, kind="ExternalOutput").ap()
    aps["new_norm"] = nc.dram_tensor("new_norm", [1, D], F32, kind="ExternalOutput").ap()

    with tile.TileContext(nc) as tc:
        _build(nc, tc, aps)
    nc.compile()
    nc.m = get_hw_module(nc.m)
    return nc


def _build(nc, tc, aps):
    q_ap, k_ap, v_ap, mems_ap, norms_ap, bmask_ap, m0row_ap = (
        aps["q"], aps["k"], aps["v"], aps["mems"], aps["norms"], aps["bmask"],
        aps["m0row"])
    out_ap, nmem_ap, nnorm_ap = aps["out_rows"], aps["nmem_shard"], aps["new_norm"]
    rg = [list(range(NC))]
    with ExitStack() as stack:
        singles = stack.enter_context(tc.tile_pool(name="singles", bufs=1))
        persist = stack.enter_context(tc.tile_pool(name="persist", bufs=1))
        dram = stack.enter_context(tc.tile_pool(name="dram", bufs=1, space="DRAM"))

        # ---------- constants ----------
        ident_f = singles.tile([P, P], F32, tag="ident_f", name="ident_f")
        make_identity(nc, ident_f)
        ident_r = singles.tile([P, P], F32R, tag="ident_r", name="ident_r")
        nc.vector.tensor_copy(ident_r, ident_f)

        ones_f = singles.tile([P, 1], F32, tag="ones_f", name="ones_f")
        nc.vector.memset(ones_f, 1.0)
        ones_r = singles.tile([P, 1], F32R, tag="ones_r", name="ones_r")
        nc.vector.tensor_copy(ones_r, ones_f)

        bmask_sb = singles.tile([P, 2], F32, tag="bmask_sb", name="bmask_sb")
        rel8 = singles.tile([1, M, 2], F32, tag="rel8", name="rel8")
        cs_sbs = [singles.tile([1, H], F32, tag=f"cs_sb{h}", name=f"cs_sb{h}")
                  for h in range(2)]
        # norms^T tiles [d-part, m]; loaded inside phase A (slow strided DMA
        # must not sit in front of the input-tile queue)
        normsT_f = [singles.tile([P, M], F32, tag=f"nTf{kd}", name=f"nTf{kd}")
                    for kd in range(KD)]
        normsT = [singles.tile([P, M], F32R, tag=f"nTr{kd}", name=f"nTr{kd}")
                  for kd in range(KD)]

        # persistent across A..C
        mem0_r = [persist.tile([P, D], F32R, tag=f"mem0_r{kd}", name=f"mem0_r{kd}")
                  for kd in range(KD)]
        sqT = [persist.tile([P, D], F32R, tag=f"sqT{st}", name=f"sqT{st}")
               for st in range(ST)]
        qnorm_r = [persist.tile([P, M], F32R, tag=f"qnorm{st}", name=f"qnorm{st}")
                   for st in range(ST)]
        c_all = [persist.tile([P, M], F32, tag=f"c_all{st}", name=f"c_all{st}")
                 for st in range(ST)]
        # m=1 eb0 weight prefetch (filled after phase A, used to bridge the RS)
        w1c = [persist.tile([P, H], F32R, tag=f"w1c{kd}", name=f"w1c{kd}")
               for kd in range(KD)]

        # collective bounce buffers
        ar1_in = dram.tile([1, 8 + D], F32, tag="ar1_in", name="ar1_in")
        ar1_out = dram.tile([1, 8 + D], F32, tag="ar1_out", name="ar1_out", addr_space="Shared")
        rs_in = dram.tile([D, D], BF16, tag="rs_in", name="rs_in")
        rs_out = dram.tile([P, D], BF16, tag="rs_out", name="rs_out")

        with ExitStack() as ab_stack:
            # tensors that live through phases A+B
            p_ab = ab_stack.enter_context(tc.tile_pool(name="p_ab", bufs=1))
            sk_nat = [p_ab.tile([P, D], F32R, tag=f"sk_nat{st}", name=f"sk_nat{st}")
                      for st in range(ST)]
            dv = [p_ab.tile([P, D], F32R, tag=f"dv{st}", name=f"dv{st}")
                  for st in range(ST)]

            # ========== phase A: sigma, transposes, qnorm/knorm, ru, dv ==========
            with tc.tile_pool(name="a_in", bufs=3) as a_in, \
                 tc.tile_pool(name="a_tmp", bufs=7) as a_tmp, \
                 tc.tile_pool(name="a_skt", bufs=2) as a_skt, \
                 tc.tile_pool(name="a_sm", bufs=4) as a_sm, \
                 tc.tile_pool(name="ps_tp", bufs=2, space="PSUM") as ps_tp, \
                 tc.tile_pool(name="ps_ru", bufs=2, space="PSUM") as ps_ru, \
                 tc.tile_pool(name="ps_qn", bufs=1, space="PSUM") as ps_qn:

                for st in range(ST):
                    rs = slice(st * P, (st + 1) * P)
                    skT = a_skt.tile([P, D], F32R, tag="skT", name="skT")
                    # sigma(x) = exp(-relu(-x)) + relu(x); relu on ACT (scale=+-1),
                    # exp on ACT, adds on DVE. Function-batched to limit ACT
                    # table swaps: all Relus, then all Exps per s-tile.
                    k_ts, q_ts, tpqs = [], [], []
                    kneg, kpos, qneg, qpos = [], [], [], []
                    for h in range(2):
                        hs = slice(h * H, (h + 1) * H)
                        k_t = a_in.tile([P, H], F32, tag="k_t", name="k_t")
                        nc.sync.dma_start(k_t, k_ap[rs, hs])
                        k_ts.append(k_t)
                        q_t = a_in.tile([P, H], F32, tag="q_t", name="q_t")
                        nc.sync.dma_start(q_t, q_ap[rs, hs])
                        q_ts.append(q_t)
                    # transposes of raw q (PE) into psum
                    for h in range(2):
                        tpq = ps_tp.tile([P, H], F32, tag="tpq", name="tpq")
                        for j in range(4):
                            nc.tensor.transpose(
                                tpq[:, j * P:(j + 1) * P],
                                q_ts[h][:, j * P:(j + 1) * P], ident_f)
                        tpqs.append(tpq)
                    # k group on ACT: relu(-x), relu(x), exp(-r)
                    for h in range(2):
                        kn = a_tmp.tile([P, H], F32, tag="tmp", name="kn")
                        nc.scalar.activation(kn, k_ts[h], AF.Relu, scale=-1.0)
                        kneg.append(kn)
                        kp = a_tmp.tile([P, H], F32, tag="tmp", name="kp")
                        nc.scalar.activation(kp, k_ts[h], AF.Relu)
                        kpos.append(kp)
                    for h in range(2):
                        hs = slice(h * H, (h + 1) * H)
                        ke = a_tmp.tile([P, H], F32, tag="tmp", name="ke")
                        nc.scalar.activation(ke, kneg[h], AF.Exp, scale=-1.0)
                        nc.vector.tensor_tensor(sk_nat[st][:, hs], ke, kpos[h],
                                                ALU.add)
                    # q group on ACT
                    for h in range(2):
                        qn = a_tmp.tile([P, H], F32, tag="tmp", name="qn")
                        nc.scalar.activation(qn, tpqs[h], AF.Relu, scale=-1.0)
                        qneg.append(qn)
                        qp = a_tmp.tile([P, H], F32, tag="tmp", name="qp")
                        nc.scalar.activation(qp, tpqs[h], AF.Relu)
                        qpos.append(qp)
                    for h in range(2):
                        hs = slice(h * H, (h + 1) * H)
                        qe = a_tmp.tile([P, H], F32, tag="tmp", name="qe")
                        nc.scalar.activation(qe, qneg[h], AF.Exp, scale=-1.0)
                        nc.vector.tensor_tensor(sqT[st][:, hs], qe, qpos[h],
                                                ALU.add)
                    # --- sigma_k transposed (PE transpose f32r, DVE copyback) ---
                    for h in range(2):
                        tps = ps_tp.tile([P, H], F32R, tag="tps", name="tps")
                        for j in range(4):
                            kd = h * 4 + j
                            nc.tensor.transpose(
                                tps[:, j * P:(j + 1) * P],
                                sk_nat[st][:, kd * P:(kd + 1) * P], ident_r)
                        nc.vector.tensor_copy(skT[:, h * H:(h + 1) * H], tps)

                    if st == 0:
                        nc.sync.dma_start(out=bmask_sb,
                                          in_=bmask_ap.to_broadcast((P, 2)))
                        for kd in range(KD):
                            with nc.allow_non_contiguous_dma(reason="norms^T"):
                                nc.sync.dma_start(
                                    normsT_f[kd],
                                    norms_ap[:, kd * P:(kd + 1) * P]
                                    .rearrange("m d -> d m"))
                            nc.vector.tensor_copy(normsT[kd], normsT_f[kd])
                        # mem0 load + round, after st0's inputs so PE starts early
                        for kd in range(KD):
                            for h in range(2):
                                hs = slice(h * H, (h + 1) * H)
                                m0f = a_tmp.tile([P, H], F32, tag="tmp", name="m0f")
                                nc.sync.dma_start(
                                    m0f, mems_ap[0, kd * P:(kd + 1) * P, hs])
                                nc.vector.tensor_copy(mem0_r[kd][:, hs], m0f)
                    if st == 1:
                        # m=1 eb0 weight prefetch (bridges the RS window in C)
                        for kd in range(KD):
                            wf = a_tmp.tile([P, H], F32, tag="tmp", name="w1pf")
                            nc.sync.dma_start(
                                wf, mems_ap[1, kd * P:(kd + 1) * P, 0:H])
                            nc.scalar.activation(w1c[kd], wf, AF.Copy)

                    # --- qnorm / knorm ---
                    qn_ps = ps_qn.tile([P, M], F32, tag="qn", name="qn_ps")
                    for kd in range(KD):
                        cs = slice(kd * P, (kd + 1) * P)
                        nc.tensor.matmul(qn_ps, sqT[st][:, cs], normsT[kd],
                                         start=(kd == 0), stop=(kd == KD - 1))
                    kn_ps = ps_qn.tile([P, M], F32, tag="kn", name="kn_ps")
                    for kd in range(KD):
                        cs = slice(kd * P, (kd + 1) * P)
                        nc.tensor.matmul(kn_ps, skT[:, cs], normsT[kd],
                                         start=(kd == 0), stop=(kd == KD - 1))
                    nc.vector.tensor_copy(qnorm_r[st], qn_ps)
                    knr = a_sm.tile([P, 1], F32, tag="knr", name="knr")
                    nc.vector.reciprocal(knr, kn_ps[:, 0:1])

                    # --- retrieved_unnorm + delta_v ---
                    for eb in range(EB):
                        es = slice(eb * H, (eb + 1) * H)
                        v_t = a_in.tile([P, H], F32, tag="v_t", name="v_t")
                        nc.sync.dma_start(v_t, v_ap[rs, es])
                        ru_ps = ps_ru.tile([P, H], F32, tag="ru", name="ru_ps")
                        for kd in range(KD):
                            cs = slice(kd * P, (kd + 1) * P)
                            nc.tensor.matmul(ru_ps, skT[:, cs], mem0_r[kd][:, es],
                                             start=(kd == 0), stop=(kd == KD - 1))
                        dvt = a_tmp.tile([P, H], F32, tag="tmp", name="dvt")
                        nc.vector.tensor_scalar_mul(dvt, ru_ps, knr)
                        nc.vector.tensor_tensor(dv[st][:, es], v_t, dvt, ALU.subtract)


            # ===== rel partials + colsum + small AllReduce (early) =====
            with tc.tile_pool(name="ps_rel", bufs=1, space="PSUM") as ps_rel, \
                 tc.tile_pool(name="ps_cs", bufs=1, space="PSUM") as ps_cs:
                rel_ps = ps_rel.tile([1, M], F32, tag="rel", name="rel_ps")
                for st in range(ST):
                    nc.tensor.matmul(rel_ps, ones_r, qnorm_r[st],
                                     start=(st == 0), stop=(st == ST - 1))
                nc.vector.tensor_tensor(
                    rel8,
                    rel_ps[0:1, :, None].to_broadcast((1, M, 2)),
                    bmask_sb[0:1, None, :].to_broadcast((1, M, 2)),
                    ALU.mult)
                nc.sync.dma_start(
                    ar1_in[0:1, 0:8].rearrange("o (f t) -> o f t", t=2), rel8)
                for h in range(2):
                    es = slice(h * H, (h + 1) * H)
                    cs_ps = ps_cs.tile([1, H], F32, tag="cs", name="cs_ps")
                    for st in range(ST):
                        nc.tensor.matmul(cs_ps, ones_r, sk_nat[st][:, es],
                                         start=(st == 0), stop=(st == ST - 1))
                    nc.vector.tensor_copy(cs_sbs[h], cs_ps)
                    nc.sync.dma_start(ar1_in[0:1, 8 + h * H:8 + (h + 1) * H],
                                      cs_sbs[h])
                nc.gpsimd.collective_compute(
                    "AllReduce", ALU.add, replica_groups=rg,
                    ins=[ar1_in.opt()], outs=[ar1_out.opt()])

            # ========== phase B: mu partials + ReduceScatter ==========
            with tc.tile_pool(name="b_stage", bufs=3) as b_stage, \
                 tc.tile_pool(name="ps_mu", bufs=2, space="PSUM") as ps_mu:
                for kd in range(KD):
                    cs_k = slice(kd * P, (kd + 1) * P)
                    for eb in range(EB):
                        es = slice(eb * H, (eb + 1) * H)
                        mu_ps = ps_mu.tile([P, H], F32, tag="mu", name="mu_ps")
                        for st in range(ST):
                            nc.tensor.matmul(mu_ps, sk_nat[st][:, cs_k],
                                             dv[st][:, es],
                                             start=(st == 0), stop=(st == ST - 1))
                        stg = b_stage.tile([P, H], BF16, tag="stg", name="stg")
                        nc.vector.tensor_copy(stg, mu_ps)
                        nc.sync.dma_start(rs_in[kd * P:(kd + 1) * P, es], stg)
                nc.gpsimd.collective_compute(
                    "ReduceScatter", ALU.add, replica_groups=rg,
                    ins=[rs_in.opt()], outs=[rs_out.opt()])

        # ========== weights from AR1 (redundant on all partitions), new_norm ====
        with tc.tile_pool(name="w_sb", bufs=1) as wp:
            g8 = wp.tile([P, M, 2], F32, tag="g8", name="g8")
            nc.sync.dma_start(
                g8, ar1_out[0:1, 0:8].rearrange("o (f t) -> o f t", t=2)
                .to_broadcast((P, M, 2)))
            t0 = wp.tile([P, M], F32, tag="t0", name="t0")
            nc.vector.tensor_scalar_mul(t0, g8[:, :, 0], bmask_sb[:, 0:1])
            t1 = wp.tile([P, M], F32, tag="t1", name="t1")
            nc.vector.tensor_scalar_mul(t1, g8[:, :, 1], bmask_sb[:, 1:2])
            rsum = wp.tile([P, M], F32, tag="rsum", name="rsum")
            nc.vector.tensor_tensor(rsum, t0, t1, ALU.add)
            mx = wp.tile([P, 1], F32, tag="mx", name="mx")
            nc.vector.tensor_reduce(mx, rsum, axis=mybir.AxisListType.X, op=ALU.max)
            negmx = wp.tile([P, 1], F32, tag="negmx", name="negmx")
            nc.vector.tensor_scalar_mul(negmx, mx, -INV_S)
            ex = wp.tile([P, M], F32, tag="ex", name="ex")
            nc.scalar.activation(ex, rsum, AF.Exp, bias=negmx, scale=INV_S)
            sm = wp.tile([P, 1], F32, tag="sm", name="sm")
            nc.vector.tensor_reduce(sm, ex, axis=mybir.AxisListType.X, op=ALU.add)
            rs_ = wp.tile([P, 1], F32, tag="rs_", name="rs_")
            nc.vector.reciprocal(rs_, sm)
            w128 = wp.tile([P, M], F32, tag="w128", name="w128")
            nc.vector.tensor_scalar_mul(w128, ex, rs_)
            for st in range(ST):
                rq = wp.tile([P, M], F32, tag=f"rq{st}", name=f"rq{st}")
                nc.vector.reciprocal(rq, qnorm_r[st].bitcast(F32))
                nc.vector.tensor_tensor(c_all[st], rq, w128, ALU.mult)
            # new_norm = norms[0] + colsum/B  (from AR1)
            csg = wp.tile([1, D], F32, tag="csg", name="csg")
            nc.sync.dma_start(csg, ar1_out[0:1, 8:8 + D])
            n0 = wp.tile([1, D], F32, tag="n0", name="n0")
            nc.sync.dma_start(n0, norms_ap[0:1, :])
            csh = wp.tile([1, D], F32, tag="csh", name="csh")
            nc.vector.tensor_scalar_mul(csh, csg, INV_B)
            nn = wp.tile([1, D], F32, tag="nn", name="nn")
            nc.vector.tensor_tensor(nn, csh, n0, ALU.add)
            nc.sync.dma_start(nnorm_ap, nn)

        # ========== phase C: retrieve matmuls + combine (m outer) ==========
        with tc.tile_pool(name="c_out", bufs=1) as c_out, \
             tc.tile_pool(name="c_wf", bufs=3) as c_wf, \
             tc.tile_pool(name="c_wr", bufs=10) as c_wr, \
             tc.tile_pool(name="c_tmp", bufs=3) as c_tmp, \
             tc.tile_pool(name="c_d", bufs=1) as c_d, \
             tc.tile_pool(name="ps_am", bufs=4, space="PSUM") as ps_am:
            out_sb = [c_out.tile([P, D], F32, tag=f"out_sb{st}", name=f"out_sb{st}")
                      for st in range(ST)]

            def mm_group_eb(m, eb, rhs_kd_tiles):
                es = slice(eb * H, (eb + 1) * H)
                for st in range(ST):
                    am_ps = ps_am.tile([P, H], F32, tag="am", name="am_ps")
                    for kd in range(KD):
                        cs = slice(kd * P, (kd + 1) * P)
                        nc.tensor.matmul(am_ps, sqT[st][:, cs], rhs_kd_tiles[kd],
                                         start=(kd == 0), stop=(kd == KD - 1))
                    if m == 0:
                        nc.vector.tensor_scalar_mul(
                            out_sb[st][:, es], am_ps, c_all[st][:, 0:1])
                    else:
                        tmp = c_tmp.tile([P, H], F32, tag="ctmp", name="ctmp")
                        nc.vector.tensor_scalar_mul(
                            tmp, am_ps, c_all[st][:, m:m + 1])
                        nc.vector.tensor_tensor(
                            out_sb[st][:, es], out_sb[st][:, es], tmp, ALU.add)
                    if m == M - 1 and eb == EB - 1:
                        nc.sync.dma_start(out_ap[st * P:(st + 1) * P, :],
                                          out_sb[st])

            def mm_group(m, rhs_tiles):
                for eb in range(EB):
                    mm_group_eb(m, eb, [rhs_tiles[kd * EB + eb] for kd in range(KD)])

            # m=0 from cached mem0_r
            mm_group(0, [mem0_r[kd][:, slice(eb * H, (eb + 1) * H)]
                         for kd in range(KD) for eb in range(EB)])
            # m=1 eb0 from prefetched cache
            mm_group_eb(1, 0, w1c)

            def stream_group(m, eb):
                es = slice(eb * H, (eb + 1) * H)
                rhs_tiles = []
                for kd in range(KD):
                    wf = c_wf.tile([P, H], F32, tag="wf", name="wf")
                    nc.sync.dma_start(wf, mems_ap[m, kd * P:(kd + 1) * P, es])
                    wr = c_wr.tile([P, H], F32R, tag="wr", name="wr")
                    nc.scalar.activation(wr, wf, AF.Copy)
                    rhs_tiles.append(wr)
                mm_group_eb(m, eb, rhs_tiles)

            stream_group(1, 1)

            # m=2,3 streamed per e-block (post-RS DMA bandwidth)
            for m in (2, 3):
                for eb in range(EB):
                    stream_group(m, eb)

            # finalize new_memory shard (needs RS result; tiny)
            g_t = c_d.tile([P, D], BF16, tag="g_t", name="g_t")
            nc.sync.dma_start(g_t, rs_out)
            m0r_t = c_d.tile([P, D], F32, tag="m0r_t", name="m0r_t")
            nc.sync.dma_start(m0r_t, m0row_ap)
            nm = c_d.tile([P, D], F32, tag="nm", name="nm")
            nc.vector.tensor_scalar_mul(nm, g_t, INV_BS)
            nm2 = c_d.tile([P, D], F32, tag="nm2", name="nm2")
            nc.vector.tensor_tensor(nm2, nm, m0r_t, ALU.add)
            nc.sync.dma_start(nmem_ap, nm2)


_CACHED = None


def _get_program():
    global _CACHED
    if _CACHED is None:
        _CACHED = build_program()
    return _CACHED


def _make_in_maps(queries, keys, values, memories, memory_norms):
    Q = np.ascontiguousarray(queries.reshape(B * S, D), dtype=np.float32)
    K = np.ascontiguousarray(keys.reshape(B * S, D), dtype=np.float32)
    V = np.ascontiguousarray(values.reshape(B * S, D), dtype=np.float32)
    mems = np.ascontiguousarray(memories, dtype=np.float32)
    norms = np.ascontiguousarray(memory_norms, dtype=np.float32)
    in_maps = []
    for c in range(NC):
        rows = slice(c * R, (c + 1) * R)
        bm = np.zeros((1, 2), np.float32)
        bm[0, c // (NC // B)] = 1.0
        in_maps.append({
            "q": Q[rows], "k": K[rows], "v": V[rows],
            "mems": mems, "norms": norms, "bmask": bm,
            "m0row": mems[0, c * P:(c + 1) * P, :],
        })
    return in_maps


def run(queries, keys, values, memories, memory_norms, trace=False, **trace_kwargs):
    nc = _get_program()
    in_maps = _make_in_maps(queries, keys, values, memories, memory_norms)
    res = bass_utils.run_bass_kernel_spmd(
        nc, in_maps, core_ids=list(range(NC)), trace=trace, **trace_kwargs)
    out = np.concatenate([res.results[c]["out_rows"] for c in range(NC)],
                         axis=0).reshape(B, S, D)
    new_memory = np.concatenate([res.results[c]["nmem_shard"] for c in range(NC)],
                                axis=0)
    new_norm = res.results[0]["new_norm"][0]
    return (out, new_memory, new_norm), res


def kernel(queries, keys, values, memories, memory_norms):
    (out, new_memory, new_norm), _ = run(
        queries, keys, values, memories, memory_norms, trace=False)
    return out, new_memory, new_norm
